# revision 1
# baseline (speedup 1.0000x reference)
"""8-core Trainium2 Bass kernel for nn_DeepSeekClone (moe_routing).

Sharding (activations kept feature-major "x^T": features on SBUF partitions,
tokens on the free axis — so every matmul consumes weights (Din,Dout) as lhsT
and x^T as rhs with zero activation transposes):
  - LN: feature-split partial stats -> 8KB AllReduce -> normalize own
    256-feature slice -> AllGather of normalized activations.
  - Attention: head-parallel (2 of 16 heads per core).
  - MoE: expert-parallel (expert c on core c), dense-per-expert with top-2
    masked gate weights (matches reference numerics exactly); combined with one
    fp32 ReduceScatter over the feature axis (= DRAM row axis in x^T layout).
  - FFN: Megatron column/row split of ffn1/ffn2 + ReduceScatter.
  - Final projection: vocab-split (4000 cols/core); host concatenates.

Precision: top-2 routing is discrete — the reference's expert choices must be
reproduced exactly. Everything feeding the two gate computations (all of layer
0, plus both gate matmuls) runs in true float32; layer 1's qkv/attention/MoE/
FFN and the final projection (which only feed final logits, not routing) run
in float32r (fp32 data at ~bf16 matmul speed on TRN2).
"""
import sys

sys.path.insert(0, "/opt/trn_rl_repo")

import numpy as np

import concourse.bass as bass
import concourse.mybir as mybir
import concourse.tile as tile
from concourse import bacc
from concourse.bass_utils import run_bass_kernel_spmd
from concourse.masks import make_identity

V, D, L, E, H, TOPK, S, B = 32000, 2048, 2, 8, 16, 2, 1024, 1
DH = D // H          # 128
DFF = 4 * D          # 8192
EPS = 1e-6
NC = 8
P = 128
FS = D // NC         # 256 features/core
FT = FS // P         # 2
HPC = H // NC        # 2 heads/core
DFS = DFF // NC      # 1024 ffn cols/core
UT = DFS // P        # 8
VS = V // NC         # 4000 vocab cols/core
KT = D // P          # 16
ST = S // P          # 8
NJ = S // 512        # 2
KORDER = list(range(16))  # half A (blocks 0-7) arrives first
F32 = mybir.dt.float32
F32R = mybir.dt.float32r
I32 = mybir.dt.int32
AX = mybir.AxisListType.X
AF = mybir.ActivationFunctionType
ALU = mybir.AluOpType
RG = [list(range(NC))]

_CACHE = {}


def _f(x):
    return np.ascontiguousarray(np.asarray(x), dtype=np.float32)


def _bcol(vec, nt):
    """(nt*128,) vector -> (128, nt) with [p, i] = vec[i*128 + p]."""
    return np.ascontiguousarray(_f(vec).reshape(nt, P).T)


def prepare_in_maps(inputs):
    tokens = np.ascontiguousarray(np.asarray(inputs["tokens"]).reshape(S), dtype=np.int32)
    embed = _f(inputs["embed"])
    qkv_w = _f(inputs["qkv_w"]); qkv_b = _f(inputs["qkv_b"])
    gate_w = _f(inputs["gate_w"]); gate_b = _f(inputs["gate_b"])
    exp_w = _f(inputs["exp_w"]); exp_b = _f(inputs["exp_b"])
    ln1_s = _f(inputs["ln1_s"]); ln1_b = _f(inputs["ln1_b"])
    ln2_s = _f(inputs["ln2_s"]); ln2_b = _f(inputs["ln2_b"])
    ffn1_w = _f(inputs["ffn1_w"]); ffn1_b = _f(inputs["ffn1_b"])
    ffn2_w = _f(inputs["ffn2_w"]); ffn2_b = _f(inputs["ffn2_b"])
    out_w = _f(inputs["out_w"]); out_b = _f(inputs["out_b"])

    tok8 = np.ascontiguousarray(tokens.reshape(ST, P))
    maps = []
    for c in range(NC):
        # core c owns feature blocks {c, c+8} (contiguous-half RS/AG mapping)
        fidx = np.concatenate([np.arange(c * P, (c + 1) * P),
                               np.arange((NC + c) * P, (NC + c + 1) * P)])
        m = {"tok": tok8, "emb": np.ascontiguousarray(embed[:, fidx])}
        for l in range(L):
            hs = [c, NC + c]
            s1 = ln1_s[l][:, None]; b1v = ln1_b[l]
            s2 = ln2_s[l][:, None]; b2v = ln2_b[l]
            wqk = np.concatenate(
                [qkv_w[l][:, 0 * D + h * DH:0 * D + (h + 1) * DH] for h in hs]
                + [qkv_w[l][:, 1 * D + h * DH:1 * D + (h + 1) * DH] for h in hs], axis=1)
            bqk = np.concatenate(
                [qkv_b[l][0 * D + h * DH:0 * D + (h + 1) * DH] for h in hs]
                + [qkv_b[l][1 * D + h * DH:1 * D + (h + 1) * DH] for h in hs])
            wv = np.concatenate(
                [qkv_w[l][:, 2 * D + h * DH:2 * D + (h + 1) * DH] for h in hs], axis=1)
            bv = np.concatenate(
                [qkv_b[l][2 * D + h * DH:2 * D + (h + 1) * DH] for h in hs])
            # fold LN1 scale/bias into qkv/v/expert weights, LN2 into ffn1
            bqk = bqk + b1v @ wqk
            wqk = s1 * wqk
            bv = bv + b1v @ wv
            wv = s1 * wv
            we_c = s1 * exp_w[l][c]
            be_c = exp_b[l][c] + b1v @ exp_w[l][c]
            w1_c = s2 * ffn1_w[l][:, c * DFS:(c + 1) * DFS]
            b1_c = ffn1_b[l][c * DFS:(c + 1) * DFS] + b2v @ ffn1_w[l][:, c * DFS:(c + 1) * DFS]
            sel = np.zeros((E, P), np.float32); sel[c, :] = 1.0
            # gate from unnormalized x: logits = rinv*(x@ws - mu*ce) + de
            ws1 = ln1_s[l][:, None] * gate_w[l]                      # (D, E)
            ce1 = ws1.sum(axis=0).reshape(E, 1)
            de1 = (ln1_b[l] @ gate_w[l] + gate_b[l]).reshape(E, 1)
            m.update({
                f"ln1s_{l}": _bcol(ln1_s[l][fidx], FT),
                f"ln1b_{l}": _bcol(ln1_b[l][fidx], FT),
                f"wqk_{l}": np.ascontiguousarray(wqk),
                f"bqk_{l}": _bcol(bqk, 2 * HPC),
                f"wv_{l}": np.ascontiguousarray(wv),
                f"bv_{l}": np.ascontiguousarray(bv.reshape(1, HPC * DH)),
                f"ws_{l}": np.ascontiguousarray(ws1),
                f"ce_{l}": np.ascontiguousarray(ce1),
                f"de_{l}": np.ascontiguousarray(de1),
                f"sel_{l}": sel,
                f"we_{l}": np.ascontiguousarray(we_c),
                f"be_{l}": _bcol(be_c, KT),
                f"ln2s_{l}": _bcol(ln2_s[l][fidx], FT),
                f"ln2b_{l}": _bcol(ln2_b[l][fidx], FT),
                f"w1_{l}": np.ascontiguousarray(w1_c),
                f"b1_{l}": _bcol(b1_c, UT),
                f"w2_{l}": np.ascontiguousarray(ffn2_w[l][c * DFS:(c + 1) * DFS, :]),
                f"b2_{l}": _bcol(ffn2_b[l][fidx], FT),
            })
        m["wo"] = np.ascontiguousarray(out_w[:, c * VS:(c + 1) * VS])
        m["bo"] = np.ascontiguousarray(out_b[c * VS:(c + 1) * VS].reshape(1, VS))
        maps.append(m)
    return maps


def _ldt(l):
    """Matmul dtype for layer l's non-gate blocks."""
    return F32 if l == 0 else F32R


def build_nc():
    nc = bacc.Bacc("TRN2", target_bir_lowering=False, debug=False, num_devices=NC)

    tok = nc.dram_tensor("tok", [ST, P], I32, kind="ExternalInput")
    emb = nc.dram_tensor("emb", [V, FS], F32, kind="ExternalInput")
    ins = {}
    for l in range(L):
        dt = _ldt(l)
        for nm, shape, d in [
            (f"ln1s_{l}", [P, FT], F32), (f"ln1b_{l}", [P, FT], F32),
            (f"wqk_{l}", [D, 4 * DH], dt), (f"bqk_{l}", [P, 4], F32),
            (f"wv_{l}", [D, 2 * DH], dt), (f"bv_{l}", [1, 2 * DH], F32),
            (f"ws_{l}", [D, E], F32), (f"ce_{l}", [E, 1], F32),
            (f"de_{l}", [E, 1], F32), (f"sel_{l}", [E, P], F32),
            (f"we_{l}", [D, D], dt), (f"be_{l}", [P, KT], F32),
            (f"ln2s_{l}", [P, FT], F32), (f"ln2b_{l}", [P, FT], F32),
            (f"w1_{l}", [D, DFS], dt), (f"b1_{l}", [P, UT], F32),
            (f"w2_{l}", [DFS, D], dt), (f"b2_{l}", [P, FT], F32),
        ]:
            ins[nm] = nc.dram_tensor(nm, shape, d, kind="ExternalInput")
    wo = nc.dram_tensor("wo", [D, VS], F32R, kind="ExternalInput")
    bo = nc.dram_tensor("bo", [1, VS], F32, kind="ExternalInput")
    out = nc.dram_tensor("out", [S, VS], F32, kind="ExternalOutput")
    import os
    if os.environ.get("KDEBUG"):
        nc._dbg = {
            "tT0": nc.dram_tensor("dbg_tT0", [P, S], F32, kind="ExternalOutput"),
            "xsl0": nc.dram_tensor("dbg_xsl0", [P, S], F32, kind="ExternalOutput"),
            "yT0l1": nc.dram_tensor("dbg_yT0l1", [P, S], F32, kind="ExternalOutput"),
            "gTl1": nc.dram_tensor("dbg_gTl1", [E, S], F32, kind="ExternalOutput"),
            "u0": nc.dram_tensor("dbg_u0", [P, S], F32, kind="ExternalOutput"),
            "yT0": nc.dram_tensor("dbg_yT0", [P, S], F32, kind="ExternalOutput"),
            "yT8": nc.dram_tensor("dbg_yT8", [P, S], F32, kind="ExternalOutput"),
            "gT": nc.dram_tensor("dbg_gT", [E, S], F32, kind="ExternalOutput"),
            "ewb": nc.dram_tensor("dbg_ewb", [P, S], F32, kind="ExternalOutput"),
            "ao0": nc.dram_tensor("dbg_ao0", [P, S], F32, kind="ExternalOutput"),
            "x20": nc.dram_tensor("dbg_x20", [P, S], F32, kind="ExternalOutput"),
        }
    else:
        nc._dbg = None

    with tile.TileContext(nc) as tc:
        _build_body(nc, tc, tok, emb, ins, wo, bo, out)
    nc.compile()
    return nc


def _build_body(nc, tc, tok, emb, ins, wo, bo, out):
    from contextlib import ExitStack

    with ExitStack() as ctx:
        cb = ctx.enter_context(tc.tile_pool(name="cb", bufs=1))
        act = ctx.enter_context(tc.tile_pool(name="act", bufs=1))
        pp = ctx.enter_context(tc.tile_pool(name="pp", bufs=1, space="PSUM"))
        dr = ctx.enter_context(tc.tile_pool(name="dr", bufs=1, space="DRAM"))

        # ---------- constants ----------
        ident_f = cb.tile([P, P], F32, name="ident_f", tag="ident_f")
        make_identity(nc, ident_f)
        ident_r = cb.tile([P, P], F32R, name="ident_r", tag="ident_r")
        nc.vector.tensor_copy(ident_r[:], ident_f[:])
        ones_cf = cb.tile([P, 1], F32, name="ones_cf", tag="ones_cf")
        nc.vector.memset(ones_cf[:], 1.0)
        ones_cr = cb.tile([P, 1], F32R, name="ones_cr", tag="ones_cr")
        nc.vector.tensor_copy(ones_cr[:], ones_cf[:])
        ones_rf = cb.tile([1, P], F32, name="ones_rf", tag="ones_rf")
        nc.vector.tensor_copy(ones_rf[:], ones_cf[:1, :].to_broadcast([1, P]))
        eps_pp = cb.tile([P, 1], F32, name="eps_pp", tag="eps_pp")
        nc.vector.memset(eps_pp[:], EPS)

        def ident(dt):
            return ident_f if dt == F32 else ident_r

        def ones_c(dt):
            return ones_cf if dt == F32 else ones_cr

        # ---------- persistent activation tiles (all fp32) ----------
        x_sl = [act.tile([P, S], F32, name=f"x_sl{fi}", tag=f"x_sl{fi}") for fi in range(FT)]
        y_sl = [act.tile([P, S], F32, name=f"y_sl{fi}", tag=f"y_sl{fi}") for fi in range(FT)]
        x2_sl = [act.tile([P, S], F32, name=f"x2_sl{fi}", tag=f"x2_sl{fi}") for fi in range(FT)]
        aoT = [act.tile([P, S], F32, name=f"aoT{h}", tag=f"aoT{h}") for h in range(HPC)]
        ew_b = act.tile([P, S], F32, name="ew_b", tag="ew_b")

        # =============== distributed layernorm (gather-first) ===============
        def ln(xs, s_in, b_in, yT, y_dst, name, l, gate_ws=None, gT=None):
            """AllGather the *unnormalized* x (contiguous halves), accumulate
            full stats (+ optional fp32 gate projection) per arriving tile,
            then normalize all 16 tiles in place (through a f32r view for
            fp32r layers) and this core's slice into y_dst.
            xs: FT local (P,S) F32 tiles = this core's feature blocks {c, c+8}.
            sf_in: full (P, KT) scale/bias params; s_in/b_in: sliced (P, FT).
            yT: 16 (P,S) F32 tiles (written: gathered then normalized)."""
            dt = _ldt(l)
            with tc.tile_pool(name=f"ln_{name}", bufs=1) as wp:
                if s_in is not None:
                    s_pp = wp.tile([P, FT], F32, name="s_pp", tag="s_pp")
                    nc.sync.dma_start(s_pp[:], s_in[:])
                    b_pp = wp.tile([P, FT], F32, name="b_pp", tag="b_pp")
                    nc.sync.dma_start(b_pp[:], b_in[:])
                if gate_ws is not None:
                    ws_t = wp.tile([P, KT * E], F32, name="ws_t", tag="ws_t")
                    nc.sync.dma_start(ws_t[:].rearrange("p (kt e) -> p kt e", e=E),
                                      gate_ws[:, :].rearrange("(kt p) e -> p kt e", p=P))
                ag_ins = [dr.tile([P, S], F32, name=f"agin{h}", tag=f"agin{h}_{name}")
                          for h in range(FT)]
                ag_outs = [dr.tile([D // FT, S], F32, name=f"agout{h}",
                                   tag=f"agout{h}_{name}", addr_space="Shared")
                           for h in range(FT)]
                for fi in range(FT):
                    nc.sync.dma_start(ag_ins[fi][:], xs[fi][:])
                    nc.gpsimd.collective_compute("AllGather", ALU.bypass, replica_groups=RG,
                                                 ins=[ag_ins[fi].opt()],
                                                 outs=[ag_outs[fi].opt()])
                # per-arriving-tile: stats (+ gate raw) accumulation, all fp32
                ps_st = [[pp.tile([1, 512], F32, name="ps_st", tag=f"b{st * NJ + nj}")
                          for nj in range(NJ)] for st in range(2)]
                if gate_ws is not None:
                    ps_g = [pp.tile([E, 512], F32, name="ps_g", tag=f"b{4 + nj}")
                            for nj in range(NJ)]
                for ki in range(KT):
                    h, r = ki // ST, ki % ST
                    nc.sync.dma_start(yT[ki][:], ag_outs[h][r * P:(r + 1) * P, :])
                    sq = wp.tile([P, S], F32, name="sq", tag="sq", bufs=3)
                    nc.scalar.activation(sq[:], yT[ki][:], AF.Square)
                    for nj in range(NJ):
                        nc.tensor.matmul(ps_st[0][nj][:], ones_cf[:],
                                         yT[ki][:, nj * 512:(nj + 1) * 512],
                                         start=(ki == 0), stop=(ki == KT - 1))
                        nc.tensor.matmul(ps_st[1][nj][:], ones_cf[:],
                                         sq[:, nj * 512:(nj + 1) * 512],
                                         start=(ki == 0), stop=(ki == KT - 1))
                        if gate_ws is not None:
                            nc.tensor.matmul(ps_g[nj][:], ws_t[:, ki * E:(ki + 1) * E],
                                             yT[ki][:, nj * 512:(nj + 1) * 512],
                                             start=(ki == 0), stop=(ki == KT - 1))
                # rows: mu, rinv
                mu_row = wp.tile([1, S], F32, name="mu_row", tag="mu_row")
                e2_row = wp.tile([1, S], F32, name="e2_row", tag="e2_row")
                for nj in range(NJ):
                    nc.scalar.mul(mu_row[:, nj * 512:(nj + 1) * 512], ps_st[0][nj][:], 1.0 / D)
                    nc.scalar.mul(e2_row[:, nj * 512:(nj + 1) * 512], ps_st[1][nj][:], 1.0 / D)
                var_row = wp.tile([1, S], F32, name="var_row", tag="var_row")
                nc.vector.tensor_mul(var_row[:], mu_row[:], mu_row[:])
                nc.vector.tensor_sub(var_row[:], e2_row[:], var_row[:])
                sd_row = wp.tile([1, S], F32, name="sd_row", tag="sd_row")
                nc.scalar.activation(sd_row[:], var_row[:], AF.Sqrt, bias=eps_pp[:1, :])
                rinv_row = wp.tile([1, S], F32, name="rinv_row", tag="rinv_row")
                nc.vector.reciprocal(rinv_row[:], sd_row[:])
                # broadcast across 128 partitions
                mu_b = wp.tile([P, S], F32, name="mu_b", tag="mu_b")
                rinv_b = wp.tile([P, S], F32, name="rinv_b", tag="rinv_b")
                for src_row, dst in ((mu_row, mu_b), (rinv_row, rinv_b)):
                    for nj in range(NJ):
                        psb = pp.tile([P, 512], F32, name="psb", tag=f"b{6 + nj}")
                        nc.tensor.matmul(psb[:], ones_rf[:],
                                         src_row[:, nj * 512:(nj + 1) * 512],
                                         start=True, stop=True)
                        nc.scalar.copy(dst[:, nj * 512:(nj + 1) * 512], psb[:])
                # gate logits gT = rinv*(graw - mu*ce) + de
                if gate_ws is not None:
                    ce_pp = wp.tile([E, 1], F32, name="ce_pp", tag="ce_pp")
                    nc.sync.dma_start(ce_pp[:], ins[f"ce_{l}"][:])
                    de_pp = wp.tile([E, 1], F32, name="de_pp", tag="de_pp")
                    nc.sync.dma_start(de_pp[:], ins[f"de_{l}"][:])
                    graw = wp.tile([E, S], F32, name="graw", tag="graw")
                    mu8 = wp.tile([E, S], F32, name="mu8", tag="mu8")
                    rinv8 = wp.tile([E, S], F32, name="rinv8", tag="rinv8")
                    for nj in range(NJ):
                        nc.scalar.copy(graw[:, nj * 512:(nj + 1) * 512], ps_g[nj][:])
                        ps8 = pp.tile([E, 512], F32, name="ps8", tag=f"b{4 + nj}")
                        nc.tensor.matmul(ps8[:], ones_rf[:1, :E],
                                         mu_row[:, nj * 512:(nj + 1) * 512],
                                         start=True, stop=True)
                        nc.scalar.copy(mu8[:, nj * 512:(nj + 1) * 512], ps8[:])
                        ps8b = pp.tile([E, 512], F32, name="ps8b", tag=f"b{4 + nj}")
                        nc.tensor.matmul(ps8b[:], ones_rf[:1, :E],
                                         rinv_row[:, nj * 512:(nj + 1) * 512],
                                         start=True, stop=True)
                        nc.scalar.copy(rinv8[:, nj * 512:(nj + 1) * 512], ps8b[:])
                    t1g = wp.tile([E, S], F32, name="t1g", tag="t1g")
                    nc.vector.scalar_tensor_tensor(t1g[:], mu8[:], ce_pp[:, 0:1], graw[:],
                                                   ALU.mult, ALU.subtract)
                    nc.vector.tensor_mul(t1g[:], t1g[:], rinv8[:])
                    nc.vector.tensor_scalar(gT[:], t1g[:], -1.0, de_pp[:, 0:1],
                                            ALU.mult, ALU.add)
                # normalize all 16 tiles in place (f32r view for fp32r layers)
                for ki in range(KT):
                    w_ap = yT[ki][:] if dt == F32 else yT[ki][:].bitcast(F32R)
                    nc.vector.tensor_sub(w_ap, yT[ki][:], mu_b[:])
                    nc.vector.tensor_mul(w_ap, yT[ki][:], rinv_b[:])
                # this core's slice (fp32, for the residual path)
                if y_dst is not None:
                    for fi in range(FT):
                        t1 = wp.tile([P, S], F32, name=f"t1_{fi}", tag=f"t1_{fi}")
                        nc.vector.tensor_sub(t1[:], xs[fi][:], mu_b[:])
                        nc.vector.tensor_mul(t1[:], t1[:], rinv_b[:])
                        nc.vector.tensor_scalar(y_dst[fi][:], t1[:], s_pp[:, fi:fi + 1],
                                                b_pp[:, fi:fi + 1], ALU.mult, ALU.add)

        # =============== embedding gather (fp32) ===============
        with tc.tile_pool(name="embp", bufs=1) as wp:
            tok_t = wp.tile([P, ST], I32, name="tok_t", tag="tok_t")
            nc.sync.dma_start(tok_t[:], tok[:, :].rearrange("c p -> p c"))
            gts = []
            for cbk in range(ST):
                g = wp.tile([P, FS], F32, name="g", tag=f"g{cbk}")
                nc.gpsimd.indirect_dma_start(
                    out=g[:], out_offset=None, in_=emb[:],
                    in_offset=bass.IndirectOffsetOnAxis(ap=tok_t[:, cbk:cbk + 1], axis=0))
                gts.append(g)
            for fi in range(FT):
                for cbk in range(ST):
                    tp = pp.tile([P, P], F32, name="tp", tag=f"b{(cbk * FT + fi) % 4}")
                    nc.tensor.transpose(tp[:], gts[cbk][:, fi * P:(fi + 1) * P], ident_f[:])
                    if (cbk + fi) % 2 == 0:
                        nc.vector.tensor_copy(x_sl[fi][:, cbk * P:(cbk + 1) * P], tp[:])
                    else:
                        nc.scalar.copy(x_sl[fi][:, cbk * P:(cbk + 1) * P], tp[:])

        gT = act.tile([E, S], F32, name="gT", tag="gT")

        # =============== transformer layers ===============
        for l in range(L):
            dt = _ldt(l)
            yT = [act.tile([P, S], F32, name=f"yT{ki}", tag=f"yT{ki}") for ki in range(KT)]

            def yslice(ki, c0, c1):
                ap = yT[ki][:, c0:c1]
                return ap if dt == F32 else ap.bitcast(F32R)

            ln(x_sl, ins[f"ln1s_{l}"], ins[f"ln1b_{l}"], yT, y_sl, f"ln1_{l}", l,
               gate_ws=ins[f"ws_{l}"], gT=gT)

            if nc._dbg is not None and l == 1:
                nc.sync.dma_start(nc._dbg["yT0l1"][:, :], yT[0][:])
                nc.sync.dma_start(nc._dbg["gTl1"][:, :], gT[:])

            # ---------- attention (2 heads) ----------
            with tc.tile_pool(name=f"attn_{l}", bufs=1) as wp:
                NHM = 2 * HPC
                bqk_pp = wp.tile([P, NHM], F32, name="bqk_pp", tag="bqk_pp")
                nc.sync.dma_start(bqk_pp[:], ins[f"bqk_{l}"][:])
                qkT = [wp.tile([P, S], dt, name=f"qkT{mi}", tag=f"qkT{mi}") for mi in range(NHM)]
                for nj in range(NJ):
                    bs = 4 * (nj % 2)
                    psq = [pp.tile([P, 512], F32, name=f"psq{mi}", tag=f"b{bs + mi}")
                           for mi in range(NHM)]
                    for t, ki in enumerate(KORDER):
                        wk = wp.tile([P, 4 * DH], dt, name="wqk_t", tag="wqk_t", bufs=3)
                        nc.sync.dma_start(wk[:], ins[f"wqk_{l}"][ki * P:(ki + 1) * P, :])
                        for mi in range(NHM):
                            nc.tensor.matmul(psq[mi][:], wk[:, mi * P:(mi + 1) * P],
                                             yslice(ki, nj * 512, (nj + 1) * 512),
                                             start=(t == 0), stop=(t == KT - 1))
                    for mi in range(NHM):
                        nc.scalar.activation(qkT[mi][:, nj * 512:(nj + 1) * 512], psq[mi][:],
                                             AF.Identity, bias=bqk_pp[:, mi:mi + 1])
                # v token-major
                bv_row = wp.tile([1, 2 * DH], F32, name="bv_row", tag="bv_row")
                nc.sync.dma_start(bv_row[:], ins[f"bv_{l}"][:])
                bv_b = wp.tile([P, 2 * DH], F32, name="bv_b", tag="bv_b")
                pbv = pp.tile([P, 2 * DH], F32, name="pbv", tag="b4")
                nc.tensor.matmul(pbv[:], ones_rf[:], bv_row[:], start=True, stop=True)
                nc.scalar.copy(bv_b[:], pbv[:])
                v_sb = [wp.tile([P, 2 * DH], dt, name=f"v_sb{mi}", tag=f"v_sb{mi}") for mi in range(ST)]
                for vg in range(2):
                    bs = 4 * (vg % 2)
                    psv = [pp.tile([P, 2 * DH], F32, name=f"psv{mi}", tag=f"b{bs + mi}")
                           for mi in range(4)]
                    for t, ki in enumerate(KORDER):
                        wk = wp.tile([P, 2 * DH], dt, name="wv_t", tag="wv_t", bufs=3)
                        nc.sync.dma_start(wk[:], ins[f"wv_{l}"][ki * P:(ki + 1) * P, :])
                        for mi in range(4):
                            tb = vg * 4 + mi
                            nc.tensor.matmul(psv[mi][:], yslice(ki, tb * P, (tb + 1) * P), wk[:],
                                             start=(t == 0), stop=(t == KT - 1))
                    for mi in range(4):
                        nc.vector.tensor_add(v_sb[vg * 4 + mi][:], psv[mi][:], bv_b[:])
                for h in range(HPC):
                    qh, kh = qkT[h], qkT[HPC + h]
                    AT = [wp.tile([P, S], dt, name=f"AT{kb}", tag=f"AT{kb}") for kb in range(ST)]
                    for qb in range(ST):
                        sc = wp.tile([P, S], F32, name="sc", tag=f"sc{qb % 2}")
                        for nj in range(NJ):
                            pss = pp.tile([P, 512], F32, name="pss", tag=f"b{(qb % 2) * 2 + nj}")
                            nc.tensor.matmul(pss[:], qh[:, qb * P:(qb + 1) * P],
                                             kh[:, nj * 512:(nj + 1) * 512],
                                             start=True, stop=True)
                            nc.scalar.mul(sc[:, nj * 512:(nj + 1) * 512], pss[:],
                                          1.0 / float(np.sqrt(DH)))
                        nmax = wp.tile([P, 1], F32, name="nmax", tag="nmax", bufs=2)
                        nc.vector.reduce_max(nmax[:], sc[:], axis=AX, negate=True)
                        pr = wp.tile([P, S], dt, name="pr", tag=f"pr{qb % 2}")
                        nc.scalar.activation(pr[:], sc[:], AF.Exp, bias=nmax[:], scale=1.0)
                        for kb in range(ST):
                            tp = pp.tile([P, P], dt, name="tpa", tag=f"b{4 + kb % 2}")
                            nc.tensor.transpose(tp[:], pr[:, kb * P:(kb + 1) * P], ident(dt)[:])
                            if kb % 2 == 0:
                                nc.vector.tensor_copy(AT[kb][:, qb * P:(qb + 1) * P], tp[:])
                            else:
                                nc.scalar.copy(AT[kb][:, qb * P:(qb + 1) * P], tp[:])
                    ao = aoT[h]
                    for nj in range(NJ):
                        po = pp.tile([P, 512], F32, name="po", tag=f"b{6 + nj}")
                        for kb in range(ST):
                            nc.tensor.matmul(po[:], v_sb[kb][:, h * DH:(h + 1) * DH],
                                             AT[kb][:, nj * 512:(nj + 1) * 512],
                                             start=(kb == 0), stop=(kb == ST - 1))
                        nc.vector.tensor_copy(ao[:, nj * 512:(nj + 1) * 512], po[:])
                    # per-token softmax denominator: column-sums of A^T
                    srow = wp.tile([1, S], F32, name="srow_a", tag="srow_a")
                    for nj in range(NJ):
                        ps_s = pp.tile([1, 512], F32, name="ps_sr", tag="b4")
                        for kb in range(ST):
                            nc.tensor.matmul(ps_s[:], ones_c(dt)[:],
                                             AT[kb][:, nj * 512:(nj + 1) * 512],
                                             start=(kb == 0), stop=(kb == ST - 1))
                        nc.scalar.copy(srow[:, nj * 512:(nj + 1) * 512], ps_s[:])
                    rrow = wp.tile([1, S], F32, name="rrow", tag="rrow")
                    nc.vector.reciprocal(rrow[:], srow[:])
                    inv_b = wp.tile([P, S], F32, name="inv_b", tag="inv_b")
                    for nj in range(NJ):
                        pbc = pp.tile([P, 512], F32, name="pbc", tag="b5")
                        nc.tensor.matmul(pbc[:], ones_rf[:], rrow[:, nj * 512:(nj + 1) * 512],
                                         start=True, stop=True)
                        nc.scalar.copy(inv_b[:, nj * 512:(nj + 1) * 512], pbc[:])
                    nc.vector.tensor_mul(ao[:], ao[:], inv_b[:])

            # ---------- gate top-2 selection (gT computed in ln) ----------
            with tc.tile_pool(name=f"gate_{l}", bufs=1) as wp:
                sel = wp.tile([E, P], F32, name="sel", tag="sel")
                nc.sync.dma_start(sel[:], ins[f"sel_{l}"][:])
                ewT = wp.tile([E, S], F32, name="ewT", tag="ewT")
                for qb in range(ST):
                    tpg = pp.tile([P, E], F32, name="tpg", tag="b1")
                    nc.tensor.transpose(tpg[:], gT[:, qb * P:(qb + 1) * P], ident_f[:E, :E])
                    gtok = wp.tile([P, E], F32, name="gtok", tag="gtok", bufs=2)
                    nc.vector.tensor_copy(gtok[:], tpg[:])
                    nm1 = wp.tile([P, 1], F32, name="nm1", tag="nm1", bufs=2)
                    nc.vector.reduce_max(nm1[:], gtok[:], axis=AX, negate=True)
                    eg = wp.tile([P, E], F32, name="eg", tag="eg", bufs=2)
                    gs = wp.tile([P, 1], F32, name="gs", tag="gs", bufs=2)
                    nc.scalar.activation(eg[:], gtok[:], AF.Exp, bias=nm1[:], accum_out=gs[:])
                    rg_ = wp.tile([P, 1], F32, name="rg_", tag="rg_", bufs=2)
                    nc.vector.reciprocal(rg_[:], gs[:])
                    p_t = wp.tile([P, E], F32, name="p_t", tag="p_t", bufs=2)
                    nc.vector.tensor_scalar_mul(p_t[:], eg[:], rg_[:])
                    m1 = wp.tile([P, 1], F32, name="m1", tag="m1", bufs=2)
                    nc.vector.reduce_max(m1[:], p_t[:], axis=AX)
                    mask = wp.tile([P, E], F32, name="mask", tag="mask", bufs=2)
                    nc.vector.tensor_scalar(mask[:], p_t[:], m1[:], None, ALU.is_equal)
                    pmask = wp.tile([P, E], F32, name="pmask", tag="pmask", bufs=2)
                    nc.vector.scalar_tensor_tensor(pmask[:], mask[:], -1e30, p_t[:],
                                                   ALU.mult, ALU.add)
                    m2 = wp.tile([P, 1], F32, name="m2", tag="m2", bufs=2)
                    nc.vector.reduce_max(m2[:], pmask[:], axis=AX)
                    gemask = wp.tile([P, E], F32, name="gemask", tag="gemask", bufs=2)
                    nc.vector.tensor_scalar(gemask[:], p_t[:], m2[:], None, ALU.is_ge)
                    ew_t = wp.tile([P, E], F32, name="ew_t", tag="ew_t", bufs=2)
                    nc.vector.tensor_mul(ew_t[:], p_t[:], gemask[:])
                    tpe = pp.tile([E, P], F32, name="tpe", tag="b2")
                    nc.tensor.transpose(tpe[:], ew_t[:], ident_f[:])
                    nc.vector.tensor_copy(ewT[:, qb * P:(qb + 1) * P], tpe[:])
                for nj in range(NJ):
                    pe_ = pp.tile([P, 512], F32, name="pe_", tag="b3")
                    nc.tensor.matmul(pe_[:], sel[:], ewT[:, nj * 512:(nj + 1) * 512],
                                     start=True, stop=True)
                    nc.scalar.copy(ew_b[:, nj * 512:(nj + 1) * 512], pe_[:])

            if nc._dbg is not None and l == 1:
                nc.sync.dma_start(nc._dbg["ewb"][:, :], ew_b[:])
                nc.sync.dma_start(nc._dbg["ao0"][:, :], aoT[0][:])

            # ---------- MoE partials + RS + combine ----------
            with tc.tile_pool(name=f"moe_{l}", bufs=1) as wp:
                be_pp = wp.tile([P, KT], F32, name="be_pp", tag="be_pp")
                nc.sync.dma_start(be_pp[:], ins[f"be_{l}"][:])
                rs_ins = [dr.tile([D // 2, S], F32, name=f"rs1in{h}", tag=f"rs1in{h}_{l}")
                          for h in range(2)]
                rs_outs = [dr.tile([P, S], F32, name=f"rs1out{h}", tag=f"rs1out{h}_{l}")
                           for h in range(2)]
                we_v = ins[f"we_{l}"]
                G = 0
                for h in range(2):
                    for nj in range(NJ):
                        for g in range(2):
                            bs = 4 * (G % 2); G += 1
                            psz = [pp.tile([P, 512], F32, name=f"psz{mi}", tag=f"b{bs + mi}")
                                   for mi in range(4)]
                            for t, ki in enumerate(KORDER):
                                wk = wp.tile([P, 512], dt, name="we_t", tag="we_t", bufs=6)
                                nc.sync.dma_start(wk[:], we_v[ki * P:(ki + 1) * P,
                                                             h * 1024 + g * 512:
                                                             h * 1024 + (g + 1) * 512])
                                for mi in range(4):
                                    nc.tensor.matmul(psz[mi][:], wk[:, mi * P:(mi + 1) * P],
                                                     yslice(ki, nj * 512, (nj + 1) * 512),
                                                     start=(t == 0), stop=(t == KT - 1))
                            for mi in range(4):
                                gm = h * 8 + g * 4 + mi
                                z = wp.tile([P, 512], F32, name="z", tag="z", bufs=4)
                                nc.vector.scalar_tensor_tensor(z[:], psz[mi][:],
                                                               be_pp[:, gm:gm + 1],
                                                               ew_b[:, nj * 512:(nj + 1) * 512],
                                                               ALU.add, ALU.mult)
                                nc.sync.dma_start(rs_ins[h][(g * 4 + mi) * P:(g * 4 + mi + 1) * P,
                                                            nj * 512:(nj + 1) * 512], z[:])
                    nc.gpsimd.collective_compute("ReduceScatter", ALU.add, replica_groups=RG,
                                                 ins=[rs_ins[h].opt()], outs=[rs_outs[h].opt()])
                for fi in range(FT):
                    r = wp.tile([P, S], F32, name=f"r1_{fi}", tag=f"r1_{fi}")
                    nc.sync.dma_start(r[:], rs_outs[fi][:])
                    nc.vector.tensor_add(r[:], r[:], aoT[fi][:])
                    nc.vector.tensor_add(x2_sl[fi][:], r[:], y_sl[fi][:])

            if nc._dbg is not None and l == 1:
                nc.sync.dma_start(nc._dbg["x20"][:, :], x2_sl[0][:])

            # ---------- LN2 (writes tT into yT tiles) ----------
            ln(x2_sl, None, None, yT, None, f"ln2_{l}", l)

            if nc._dbg is not None and l == 1:
                nc.sync.dma_start(nc._dbg["tT0"][:, :], yT[0][:].bitcast(F32))

            # ---------- FFN + RS ----------
            with tc.tile_pool(name=f"ffn_{l}", bufs=1) as wp:
                b1_pp = wp.tile([P, UT], F32, name="b1_pp", tag="b1_pp")
                nc.sync.dma_start(b1_pp[:], ins[f"b1_{l}"][:])
                b2_pp = wp.tile([P, FT], F32, name="b2_pp", tag="b2_pp")
                nc.sync.dma_start(b2_pp[:], ins[f"b2_{l}"][:])
                rs2_ins = [dr.tile([D // 2, S], F32, name=f"rs2in{h}", tag=f"rs2in{h}_{l}")
                           for h in range(2)]
                rs2_outs = [dr.tile([P, S], F32, name=f"rs2out{h}", tag=f"rs2out{h}_{l}")
                            for h in range(2)]
                # ffn1 -> u (full-S tiles)
                u = [wp.tile([P, S], dt, name=f"u{mi}", tag=f"u{mi}") for mi in range(UT)]
                G = 0
                for nj in range(NJ):
                    for g in range(2):
                        bs = 4 * (G % 2); G += 1
                        psu = [pp.tile([P, 512], F32, name=f"psu{mi}", tag=f"b{bs + mi}")
                               for mi in range(4)]
                        for t, ki in enumerate(KORDER):
                            wk = wp.tile([P, 512], dt, name="w1_t", tag="w1_t", bufs=4)
                            nc.sync.dma_start(wk[:], ins[f"w1_{l}"][ki * P:(ki + 1) * P,
                                                                    g * 512:(g + 1) * 512])
                            for mi in range(4):
                                nc.tensor.matmul(psu[mi][:], wk[:, mi * P:(mi + 1) * P],
                                                 yslice(ki, nj * 512, (nj + 1) * 512),
                                                 start=(t == 0), stop=(t == KT - 1))
                        for mi in range(4):
                            um = g * 4 + mi
                            nc.scalar.activation(u[um][:, nj * 512:(nj + 1) * 512], psu[mi][:],
                                                 AF.Gelu_apprx_tanh, bias=b1_pp[:, um:um + 1])
                # ffn2: halves over output features, split RS
                w2_v = ins[f"w2_{l}"]
                for h in range(2):
                    for nj in range(NJ):
                        for g in range(2):
                            bs = 4 * (G % 2); G += 1
                            psf = [pp.tile([P, 512], F32, name=f"psf{mi}", tag=f"b{bs + mi}")
                                   for mi in range(4)]
                            for ki in range(UT):
                                wk2 = wp.tile([P, 512], dt, name="w2_t", tag="w2_t", bufs=6)
                                nc.sync.dma_start(wk2[:], w2_v[ki * P:(ki + 1) * P,
                                                               h * 1024 + g * 512:
                                                               h * 1024 + (g + 1) * 512])
                                for mi in range(4):
                                    nc.tensor.matmul(psf[mi][:], wk2[:, mi * P:(mi + 1) * P],
                                                     u[ki][:, nj * 512:(nj + 1) * 512],
                                                     start=(ki == 0), stop=(ki == UT - 1))
                            for mi in range(4):
                                zf = wp.tile([P, 512], F32, name="zf", tag="zf", bufs=4)
                                nc.scalar.copy(zf[:], psf[mi][:])
                                nc.sync.dma_start(rs2_ins[h][(g * 4 + mi) * P:(g * 4 + mi + 1) * P,
                                                             nj * 512:(nj + 1) * 512], zf[:])
                    nc.gpsimd.collective_compute("ReduceScatter", ALU.add, replica_groups=RG,
                                                 ins=[rs2_ins[h].opt()], outs=[rs2_outs[h].opt()])
                for fi in range(FT):
                    r2 = wp.tile([P, S], F32, name=f"r2_{fi}", tag=f"r2_{fi}")
                    nc.sync.dma_start(r2[:], rs2_outs[fi][:])
                    nc.vector.tensor_scalar_add(x_sl[fi][:], r2[:], b2_pp[:, fi:fi + 1])
                if nc._dbg is not None and l == 1:
                    nc.sync.dma_start(nc._dbg["xsl0"][:, :], x_sl[1][:])

        # =============== final projection (vocab-split, fp32r) ===============
        with tc.tile_pool(name="finp", bufs=1) as wp:
            ag_ins = [dr.tile([P, S], F32, name=f"agf_in{h}", tag=f"agf_in{h}")
                      for h in range(FT)]
            ag_outs = [dr.tile([D // FT, S], F32, name=f"agf_out{h}", tag=f"agf_out{h}",
                               addr_space="Shared") for h in range(FT)]
            for fi in range(FT):
                nc.sync.dma_start(ag_ins[fi][:], x_sl[fi][:])
                nc.gpsimd.collective_compute("AllGather", ALU.bypass, replica_groups=RG,
                                             ins=[ag_ins[fi].opt()], outs=[ag_outs[fi].opt()])
            xfT = [act.tile([P, S], F32R, name=f"xfT{ki}", tag=f"yT{ki}") for ki in range(KT)]
            for ki in KORDER:
                h, r = ki // ST, ki % ST
                nc.gpsimd.dma_start(xfT[ki][:], ag_outs[h][r * P:(r + 1) * P, :])
            if nc._dbg is not None:
                nc.sync.dma_start(nc._dbg["yT0"][:, :], xfT[0][:].bitcast(F32))
                nc.sync.dma_start(nc._dbg["yT8"][:, :], xfT[8][:].bitcast(F32))
                nc.sync.dma_start(nc._dbg["u0"][:, :], ag_outs[1][0:P, :])
            bo_row = wp.tile([1, VS], F32, name="bo_row", tag="bo_row")
            nc.sync.dma_start(bo_row[:], bo[:])
            ob_b = wp.tile([P, VS], F32, name="ob_b", tag="ob_b")
            for vb in range(8):
                pb = pp.tile([P, 500], F32, name="pb", tag=f"b{vb}")
                nc.tensor.matmul(pb[:], ones_rf[:], bo_row[:, vb * 500:(vb + 1) * 500],
                                 start=True, stop=True)
                nc.scalar.copy(ob_b[:, vb * 500:(vb + 1) * 500], pb[:])
            G = 0
            for nj in range(8):
                wks = []
                for ki in range(KT):
                    wk = wp.tile([P, 500], F32R, name="wo_k", tag="wo_k", bufs=KT + 2)
                    nc.sync.dma_start(wk[:], wo[ki * P:(ki + 1) * P, nj * 500:(nj + 1) * 500])
                    wks.append(wk)
                for g in range(2):
                    bs = 4 * (G % 2); G += 1
                    psums = [pp.tile([P, 500], F32, name=f"po{mi}", tag=f"b{bs + mi}")
                             for mi in range(4)]
                    for t, ki in enumerate(KORDER):
                        for mi in range(4):
                            tb = g * 4 + mi
                            nc.tensor.matmul(psums[mi][:], xfT[ki][:, tb * P:(tb + 1) * P],
                                             wks[ki][:], start=(t == 0), stop=(t == KT - 1))
                    for mi in range(4):
                        tb = g * 4 + mi
                        lo = wp.tile([P, 500], F32, name="lo", tag="lo", bufs=4)
                        nc.vector.tensor_add(lo[:], psums[mi][:], ob_b[:, nj * 500:(nj + 1) * 500])
                        nc.sync.dma_start(out[tb * P:(tb + 1) * P, nj * 500:(nj + 1) * 500], lo[:])


def kernel(**inputs):
    if "nc" not in _CACHE:
        _CACHE["nc"] = build_nc()
    nc = _CACHE["nc"]
    in_maps = prepare_in_maps(inputs)
    r = run_bass_kernel_spmd(nc, in_maps, core_ids=list(range(NC)), trace=False)
    logits = np.concatenate([r.results[c]["out"] for c in range(NC)], axis=1)
    return logits.reshape(B, S, V).astype(np.float32)



# revision 2
# speedup vs baseline: 1.3015x; 1.3015x over previous
"""8-core Trainium2 Bass kernel for nn_DeepSeekClone (moe_routing).

Sharding (activations kept feature-major "x^T": features on SBUF partitions,
tokens on the free axis — so every matmul consumes weights (Din,Dout) as lhsT
and x^T as rhs with zero activation transposes):
  - LN: feature-split partial stats -> 8KB AllReduce -> normalize own
    256-feature slice -> AllGather of normalized activations.
  - Attention: head-parallel (2 of 16 heads per core).
  - MoE: expert-parallel (expert c on core c), dense-per-expert with top-2
    masked gate weights (matches reference numerics exactly); combined with one
    fp32 ReduceScatter over the feature axis (= DRAM row axis in x^T layout).
  - FFN: Megatron column/row split of ffn1/ffn2 + ReduceScatter.
  - Final projection: vocab-split (4000 cols/core); host concatenates.

Precision: top-2 routing is discrete — the reference's expert choices must be
reproduced exactly. Everything feeding the two gate computations (all of layer
0, plus both gate matmuls) runs in true float32; layer 1's qkv/attention/MoE/
FFN and the final projection (which only feed final logits, not routing) run
in float32r (fp32 data at ~bf16 matmul speed on TRN2).
"""
import sys

sys.path.insert(0, "/opt/trn_rl_repo")

import numpy as np

import concourse.bass as bass
import concourse.mybir as mybir
import concourse.tile as tile
from concourse import bacc
from concourse.bass_utils import run_bass_kernel_spmd
from concourse.masks import make_identity

V, D, L, E, H, TOPK, S, B = 32000, 2048, 2, 8, 16, 2, 1024, 1
DH = D // H          # 128
DFF = 4 * D          # 8192
EPS = 1e-6
NC = 8
P = 128
FS = D // NC         # 256 features/core
FT = FS // P         # 2
HPC = H // NC        # 2 heads/core
DFS = DFF // NC      # 1024 ffn cols/core
UT = DFS // P        # 8
VS = V // NC         # 4000 vocab cols/core
KT = D // P          # 16
ST = S // P          # 8
NJ = S // 512        # 2
KORDER = list(range(16))  # half A (blocks 0-7) arrives first
F32 = mybir.dt.float32
F32R = mybir.dt.float32r
I32 = mybir.dt.int32
AX = mybir.AxisListType.X
AF = mybir.ActivationFunctionType
ALU = mybir.AluOpType
RG = [list(range(NC))]

_CACHE = {}


def _f(x):
    return np.ascontiguousarray(np.asarray(x), dtype=np.float32)


def _bcol(vec, nt):
    """(nt*128,) vector -> (128, nt) with [p, i] = vec[i*128 + p]."""
    return np.ascontiguousarray(_f(vec).reshape(nt, P).T)


def prepare_in_maps(inputs):
    tokens = np.ascontiguousarray(np.asarray(inputs["tokens"]).reshape(S), dtype=np.int32)
    embed = _f(inputs["embed"])
    qkv_w = _f(inputs["qkv_w"]); qkv_b = _f(inputs["qkv_b"])
    gate_w = _f(inputs["gate_w"]); gate_b = _f(inputs["gate_b"])
    exp_w = _f(inputs["exp_w"]); exp_b = _f(inputs["exp_b"])
    ln1_s = _f(inputs["ln1_s"]); ln1_b = _f(inputs["ln1_b"])
    ln2_s = _f(inputs["ln2_s"]); ln2_b = _f(inputs["ln2_b"])
    ffn1_w = _f(inputs["ffn1_w"]); ffn1_b = _f(inputs["ffn1_b"])
    ffn2_w = _f(inputs["ffn2_w"]); ffn2_b = _f(inputs["ffn2_b"])
    out_w = _f(inputs["out_w"]); out_b = _f(inputs["out_b"])

    tok8 = np.ascontiguousarray(tokens.reshape(ST, P))
    maps = []
    for c in range(NC):
        # core c owns feature blocks {c, c+8} (contiguous-half RS/AG mapping)
        fidx = np.concatenate([np.arange(c * P, (c + 1) * P),
                               np.arange((NC + c) * P, (NC + c + 1) * P)])
        m = {"tok": tok8, "emb": np.ascontiguousarray(embed[:, fidx])}
        for l in range(L):
            hs = [c, NC + c]
            s1 = ln1_s[l][:, None]; b1v = ln1_b[l]
            s2 = ln2_s[l][:, None]; b2v = ln2_b[l]
            wqk = np.concatenate(
                [qkv_w[l][:, 0 * D + h * DH:0 * D + (h + 1) * DH] for h in hs]
                + [qkv_w[l][:, 1 * D + h * DH:1 * D + (h + 1) * DH] for h in hs], axis=1)
            bqk = np.concatenate(
                [qkv_b[l][0 * D + h * DH:0 * D + (h + 1) * DH] for h in hs]
                + [qkv_b[l][1 * D + h * DH:1 * D + (h + 1) * DH] for h in hs])
            wv = np.concatenate(
                [qkv_w[l][:, 2 * D + h * DH:2 * D + (h + 1) * DH] for h in hs], axis=1)
            bv = np.concatenate(
                [qkv_b[l][2 * D + h * DH:2 * D + (h + 1) * DH] for h in hs])
            # fold LN1 scale/bias into qkv/v/expert weights, LN2 into ffn1
            bqk = bqk + b1v @ wqk
            wqk = s1 * wqk
            bv = bv + b1v @ wv
            wv = s1 * wv
            we_c = s1 * exp_w[l][c]
            be_c = exp_b[l][c] + b1v @ exp_w[l][c]
            w1_c = s2 * ffn1_w[l][:, c * DFS:(c + 1) * DFS]
            b1_c = ffn1_b[l][c * DFS:(c + 1) * DFS] + b2v @ ffn1_w[l][:, c * DFS:(c + 1) * DFS]
            sel = np.zeros((E, P), np.float32); sel[c, :] = 1.0
            # gate from unnormalized x: logits = rinv*(x@ws - mu*ce) + de
            ws1 = ln1_s[l][:, None] * gate_w[l]                      # (D, E)
            ce1 = ws1.sum(axis=0).reshape(E, 1)
            de1 = (ln1_b[l] @ gate_w[l] + gate_b[l]).reshape(E, 1)
            m.update({
                f"ln1s_{l}": _bcol(ln1_s[l][fidx], FT),
                f"ln1b_{l}": _bcol(ln1_b[l][fidx], FT),
                f"wqk_{l}": np.ascontiguousarray(wqk),
                f"bqk_{l}": _bcol(bqk, 2 * HPC),
                f"wv_{l}": np.ascontiguousarray(wv),
                f"bv_{l}": np.ascontiguousarray(bv.reshape(1, HPC * DH)),
                f"ws_{l}": np.ascontiguousarray(ws1),
                f"ce_{l}": np.ascontiguousarray(ce1),
                f"de_{l}": np.ascontiguousarray(de1),
                f"sel_{l}": sel,
                f"we_{l}": np.ascontiguousarray(we_c),
                f"be_{l}": _bcol(be_c, KT),
                f"ln2s_{l}": _bcol(ln2_s[l][fidx], FT),
                f"ln2b_{l}": _bcol(ln2_b[l][fidx], FT),
                f"w1_{l}": np.ascontiguousarray(w1_c),
                f"b1_{l}": _bcol(b1_c, UT),
                f"w2_{l}": np.ascontiguousarray(ffn2_w[l][c * DFS:(c + 1) * DFS, :]),
                f"b2_{l}": _bcol(ffn2_b[l][fidx], FT),
            })
        m["wo"] = np.ascontiguousarray(out_w[:, c * VS:(c + 1) * VS])
        m["bo"] = np.ascontiguousarray(out_b[c * VS:(c + 1) * VS].reshape(1, VS))
        maps.append(m)
    return maps


def _ldt(l):
    """Matmul dtype for layer l's non-gate blocks.

    float32r: fp32 data at ~4x fp32 matmul speed (1 cycle/row when free
    dim >= 256). The routing-critical paths (gate matmuls, LN stats) stay
    true float32; fp32r's PE rounding only perturbs non-gate activations
    (~3e-4 rel), far inside the 2e-2 budget."""
    return F32R


def build_nc():
    nc = bacc.Bacc("TRN2", target_bir_lowering=False, debug=False, num_devices=NC)

    tok = nc.dram_tensor("tok", [ST, P], I32, kind="ExternalInput")
    emb = nc.dram_tensor("emb", [V, FS], F32, kind="ExternalInput")
    ins = {}
    for l in range(L):
        dt = _ldt(l)
        for nm, shape, d in [
            (f"ln1s_{l}", [P, FT], F32), (f"ln1b_{l}", [P, FT], F32),
            (f"wqk_{l}", [D, 4 * DH], dt), (f"bqk_{l}", [P, 4], F32),
            (f"wv_{l}", [D, 2 * DH], dt), (f"bv_{l}", [1, 2 * DH], F32),
            (f"ws_{l}", [D, E], F32), (f"ce_{l}", [E, 1], F32),
            (f"de_{l}", [E, 1], F32), (f"sel_{l}", [E, P], F32),
            (f"we_{l}", [D, D], dt), (f"be_{l}", [P, KT], F32),
            (f"ln2s_{l}", [P, FT], F32), (f"ln2b_{l}", [P, FT], F32),
            (f"w1_{l}", [D, DFS], dt), (f"b1_{l}", [P, UT], F32),
            (f"w2_{l}", [DFS, D], dt), (f"b2_{l}", [P, FT], F32),
        ]:
            ins[nm] = nc.dram_tensor(nm, shape, d, kind="ExternalInput")
    wo = nc.dram_tensor("wo", [D, VS], F32R, kind="ExternalInput")
    bo = nc.dram_tensor("bo", [1, VS], F32, kind="ExternalInput")
    out = nc.dram_tensor("out", [S, VS], F32, kind="ExternalOutput")
    import os
    if os.environ.get("KDEBUG"):
        nc._dbg = {
            "tT0": nc.dram_tensor("dbg_tT0", [P, S], F32, kind="ExternalOutput"),
            "xsl0": nc.dram_tensor("dbg_xsl0", [P, S], F32, kind="ExternalOutput"),
            "yT0l1": nc.dram_tensor("dbg_yT0l1", [P, S], F32, kind="ExternalOutput"),
            "gTl1": nc.dram_tensor("dbg_gTl1", [E, S], F32, kind="ExternalOutput"),
            "u0": nc.dram_tensor("dbg_u0", [P, S], F32, kind="ExternalOutput"),
            "yT0": nc.dram_tensor("dbg_yT0", [P, S], F32, kind="ExternalOutput"),
            "yT8": nc.dram_tensor("dbg_yT8", [P, S], F32, kind="ExternalOutput"),
            "gT": nc.dram_tensor("dbg_gT", [E, S], F32, kind="ExternalOutput"),
            "ewb": nc.dram_tensor("dbg_ewb", [P, S], F32, kind="ExternalOutput"),
            "ao0": nc.dram_tensor("dbg_ao0", [P, S], F32, kind="ExternalOutput"),
            "x20": nc.dram_tensor("dbg_x20", [P, S], F32, kind="ExternalOutput"),
        }
    else:
        nc._dbg = None

    with tile.TileContext(nc) as tc:
        _build_body(nc, tc, tok, emb, ins, wo, bo, out)
    nc.compile()
    return nc


def _build_body(nc, tc, tok, emb, ins, wo, bo, out):
    from contextlib import ExitStack

    with ExitStack() as ctx:
        cb = ctx.enter_context(tc.tile_pool(name="cb", bufs=1))
        act = ctx.enter_context(tc.tile_pool(name="act", bufs=1))
        pp = ctx.enter_context(tc.tile_pool(name="pp", bufs=1, space="PSUM"))
        dr = ctx.enter_context(tc.tile_pool(name="dr", bufs=1, space="DRAM"))

        # ---------- constants ----------
        ident_f = cb.tile([P, P], F32, name="ident_f", tag="ident_f")
        make_identity(nc, ident_f)
        ident_r = cb.tile([P, P], F32R, name="ident_r", tag="ident_r")
        nc.vector.tensor_copy(ident_r[:], ident_f[:])
        ones_cf = cb.tile([P, 1], F32, name="ones_cf", tag="ones_cf")
        nc.vector.memset(ones_cf[:], 1.0)
        ones_cr = cb.tile([P, 1], F32R, name="ones_cr", tag="ones_cr")
        nc.vector.tensor_copy(ones_cr[:], ones_cf[:])
        ones_rf = cb.tile([1, P], F32, name="ones_rf", tag="ones_rf")
        nc.vector.tensor_copy(ones_rf[:], ones_cf[:1, :].to_broadcast([1, P]))
        eps_pp = cb.tile([P, 1], F32, name="eps_pp", tag="eps_pp")
        nc.vector.memset(eps_pp[:], EPS)

        def ident(dt):
            return ident_f if dt == F32 else ident_r

        def ones_c(dt):
            return ones_cf if dt == F32 else ones_cr

        # ---------- persistent activation tiles (all fp32) ----------
        x_sl = [act.tile([P, S], F32, name=f"x_sl{fi}", tag=f"x_sl{fi}") for fi in range(FT)]
        y_sl = [act.tile([P, S], F32, name=f"y_sl{fi}", tag=f"y_sl{fi}") for fi in range(FT)]
        x2_sl = [act.tile([P, S], F32, name=f"x2_sl{fi}", tag=f"x2_sl{fi}") for fi in range(FT)]
        aoT = [act.tile([P, S], F32, name=f"aoT{h}", tag=f"aoT{h}") for h in range(HPC)]
        ew_b = act.tile([P, S], F32, name="ew_b", tag="ew_b")

        # =============== distributed layernorm (gather-first) ===============
        def ln(xs, s_in, b_in, yT, y_dst, name, l, gate_ws=None, gT=None):
            """AllGather the *unnormalized* x (contiguous halves), accumulate
            full stats (+ optional fp32 gate projection) per arriving tile,
            then normalize all 16 tiles in place (through a f32r view for
            fp32r layers) and this core's slice into y_dst.
            xs: FT local (P,S) F32 tiles = this core's feature blocks {c, c+8}.
            sf_in: full (P, KT) scale/bias params; s_in/b_in: sliced (P, FT).
            yT: 16 (P,S) F32 tiles (written: gathered then normalized)."""
            dt = _ldt(l)
            with tc.tile_pool(name=f"ln_{name}", bufs=1) as wp:
                if s_in is not None:
                    s_pp = wp.tile([P, FT], F32, name="s_pp", tag="s_pp")
                    nc.sync.dma_start(s_pp[:], s_in[:])
                    b_pp = wp.tile([P, FT], F32, name="b_pp", tag="b_pp")
                    nc.sync.dma_start(b_pp[:], b_in[:])
                if gate_ws is not None:
                    ws_t = wp.tile([P, KT * E], F32, name="ws_t", tag="ws_t")
                    nc.sync.dma_start(ws_t[:].rearrange("p (kt e) -> p kt e", e=E),
                                      gate_ws[:, :].rearrange("(kt p) e -> p kt e", p=P))
                ag_ins = [dr.tile([P, S], F32, name=f"agin{h}", tag=f"agin{h}_{name}")
                          for h in range(FT)]
                ag_outs = [dr.tile([D // FT, S], F32, name=f"agout{h}",
                                   tag=f"agout{h}_{name}", addr_space="Shared")
                           for h in range(FT)]
                for fi in range(FT):
                    nc.sync.dma_start(ag_ins[fi][:], xs[fi][:])
                    nc.gpsimd.collective_compute("AllGather", ALU.bypass, replica_groups=RG,
                                                 ins=[ag_ins[fi].opt()],
                                                 outs=[ag_outs[fi].opt()])
                # per-arriving-tile: stats (+ gate raw) accumulation, all fp32
                ps_st = [[pp.tile([1, 512], F32, name="ps_st", tag=f"b{st * NJ + nj}")
                          for nj in range(NJ)] for st in range(2)]
                if gate_ws is not None:
                    ps_g = [pp.tile([E, 512], F32, name="ps_g", tag=f"b{4 + nj}")
                            for nj in range(NJ)]
                for ki in range(KT):
                    h, r = ki // ST, ki % ST
                    nc.sync.dma_start(yT[ki][:], ag_outs[h][r * P:(r + 1) * P, :])
                    sq = wp.tile([P, S], F32, name="sq", tag="sq", bufs=3)
                    nc.scalar.activation(sq[:], yT[ki][:], AF.Square)
                    for nj in range(NJ):
                        nc.tensor.matmul(ps_st[0][nj][:], ones_cf[:],
                                         yT[ki][:, nj * 512:(nj + 1) * 512],
                                         start=(ki == 0), stop=(ki == KT - 1))
                        nc.tensor.matmul(ps_st[1][nj][:], ones_cf[:],
                                         sq[:, nj * 512:(nj + 1) * 512],
                                         start=(ki == 0), stop=(ki == KT - 1))
                        if gate_ws is not None:
                            nc.tensor.matmul(ps_g[nj][:], ws_t[:, ki * E:(ki + 1) * E],
                                             yT[ki][:, nj * 512:(nj + 1) * 512],
                                             start=(ki == 0), stop=(ki == KT - 1))
                # rows: mu, rinv
                mu_row = wp.tile([1, S], F32, name="mu_row", tag="mu_row")
                e2_row = wp.tile([1, S], F32, name="e2_row", tag="e2_row")
                for nj in range(NJ):
                    nc.scalar.mul(mu_row[:, nj * 512:(nj + 1) * 512], ps_st[0][nj][:], 1.0 / D)
                    nc.scalar.mul(e2_row[:, nj * 512:(nj + 1) * 512], ps_st[1][nj][:], 1.0 / D)
                var_row = wp.tile([1, S], F32, name="var_row", tag="var_row")
                nc.vector.tensor_mul(var_row[:], mu_row[:], mu_row[:])
                nc.vector.tensor_sub(var_row[:], e2_row[:], var_row[:])
                sd_row = wp.tile([1, S], F32, name="sd_row", tag="sd_row")
                nc.scalar.activation(sd_row[:], var_row[:], AF.Sqrt, bias=eps_pp[:1, :])
                rinv_row = wp.tile([1, S], F32, name="rinv_row", tag="rinv_row")
                nc.vector.reciprocal(rinv_row[:], sd_row[:])
                # broadcast across 128 partitions
                mu_b = wp.tile([P, S], F32, name="mu_b", tag="mu_b")
                rinv_b = wp.tile([P, S], F32, name="rinv_b", tag="rinv_b")
                for src_row, dst in ((mu_row, mu_b), (rinv_row, rinv_b)):
                    for nj in range(NJ):
                        psb = pp.tile([P, 512], F32, name="psb", tag=f"b{6 + nj}")
                        nc.tensor.matmul(psb[:], ones_rf[:],
                                         src_row[:, nj * 512:(nj + 1) * 512],
                                         start=True, stop=True)
                        nc.scalar.copy(dst[:, nj * 512:(nj + 1) * 512], psb[:])
                # gate logits gT = rinv*(graw - mu*ce) + de
                if gate_ws is not None:
                    ce_pp = wp.tile([E, 1], F32, name="ce_pp", tag="ce_pp")
                    nc.sync.dma_start(ce_pp[:], ins[f"ce_{l}"][:])
                    de_pp = wp.tile([E, 1], F32, name="de_pp", tag="de_pp")
                    nc.sync.dma_start(de_pp[:], ins[f"de_{l}"][:])
                    graw = wp.tile([E, S], F32, name="graw", tag="graw")
                    mu8 = wp.tile([E, S], F32, name="mu8", tag="mu8")
                    rinv8 = wp.tile([E, S], F32, name="rinv8", tag="rinv8")
                    for nj in range(NJ):
                        nc.scalar.copy(graw[:, nj * 512:(nj + 1) * 512], ps_g[nj][:])
                        ps8 = pp.tile([E, 512], F32, name="ps8", tag=f"b{4 + nj}")
                        nc.tensor.matmul(ps8[:], ones_rf[:1, :E],
                                         mu_row[:, nj * 512:(nj + 1) * 512],
                                         start=True, stop=True)
                        nc.scalar.copy(mu8[:, nj * 512:(nj + 1) * 512], ps8[:])
                        ps8b = pp.tile([E, 512], F32, name="ps8b", tag=f"b{4 + nj}")
                        nc.tensor.matmul(ps8b[:], ones_rf[:1, :E],
                                         rinv_row[:, nj * 512:(nj + 1) * 512],
                                         start=True, stop=True)
                        nc.scalar.copy(rinv8[:, nj * 512:(nj + 1) * 512], ps8b[:])
                    t1g = wp.tile([E, S], F32, name="t1g", tag="t1g")
                    nc.vector.scalar_tensor_tensor(t1g[:], mu8[:], ce_pp[:, 0:1], graw[:],
                                                   ALU.mult, ALU.subtract)
                    nc.vector.tensor_mul(t1g[:], t1g[:], rinv8[:])
                    nc.vector.tensor_scalar(gT[:], t1g[:], -1.0, de_pp[:, 0:1],
                                            ALU.mult, ALU.add)
                # normalize all 16 tiles in place (f32r view for fp32r layers)
                for ki in range(KT):
                    w_ap = yT[ki][:] if dt == F32 else yT[ki][:].bitcast(F32R)
                    nc.vector.tensor_sub(w_ap, yT[ki][:], mu_b[:])
                    nc.vector.tensor_mul(w_ap, yT[ki][:], rinv_b[:])
                # this core's slice (fp32, for the residual path)
                if y_dst is not None:
                    for fi in range(FT):
                        t1 = wp.tile([P, S], F32, name=f"t1_{fi}", tag=f"t1_{fi}")
                        nc.vector.tensor_sub(t1[:], xs[fi][:], mu_b[:])
                        nc.vector.tensor_mul(t1[:], t1[:], rinv_b[:])
                        nc.vector.tensor_scalar(y_dst[fi][:], t1[:], s_pp[:, fi:fi + 1],
                                                b_pp[:, fi:fi + 1], ALU.mult, ALU.add)

        # =============== embedding gather (fp32) ===============
        with tc.tile_pool(name="embp", bufs=1) as wp:
            tok_t = wp.tile([P, ST], I32, name="tok_t", tag="tok_t")
            nc.sync.dma_start(tok_t[:], tok[:, :].rearrange("c p -> p c"))
            gts = []
            for cbk in range(ST):
                g = wp.tile([P, FS], F32, name="g", tag=f"g{cbk}")
                nc.gpsimd.indirect_dma_start(
                    out=g[:], out_offset=None, in_=emb[:],
                    in_offset=bass.IndirectOffsetOnAxis(ap=tok_t[:, cbk:cbk + 1], axis=0))
                gts.append(g)
            for fi in range(FT):
                for cbk in range(ST):
                    tp = pp.tile([P, P], F32, name="tp", tag=f"b{(cbk * FT + fi) % 4}")
                    nc.tensor.transpose(tp[:], gts[cbk][:, fi * P:(fi + 1) * P], ident_f[:])
                    if (cbk + fi) % 2 == 0:
                        nc.vector.tensor_copy(x_sl[fi][:, cbk * P:(cbk + 1) * P], tp[:])
                    else:
                        nc.scalar.copy(x_sl[fi][:, cbk * P:(cbk + 1) * P], tp[:])

        gT = act.tile([E, S], F32, name="gT", tag="gT")

        # =============== transformer layers ===============
        for l in range(L):
            dt = _ldt(l)
            yT = [act.tile([P, S], F32, name=f"yT{ki}", tag=f"yT{ki}") for ki in range(KT)]

            def yslice(ki, c0, c1):
                ap = yT[ki][:, c0:c1]
                return ap if dt == F32 else ap.bitcast(F32R)

            ln(x_sl, ins[f"ln1s_{l}"], ins[f"ln1b_{l}"], yT, y_sl, f"ln1_{l}", l,
               gate_ws=ins[f"ws_{l}"], gT=gT)

            if nc._dbg is not None and l == 1:
                nc.sync.dma_start(nc._dbg["yT0l1"][:, :], yT[0][:])
                nc.sync.dma_start(nc._dbg["gTl1"][:, :], gT[:])

            # ---------- attention (2 heads) ----------
            with tc.tile_pool(name=f"attn_{l}", bufs=1) as wp:
                NHM = 2 * HPC
                bqk_pp = wp.tile([P, NHM], F32, name="bqk_pp", tag="bqk_pp")
                nc.sync.dma_start(bqk_pp[:], ins[f"bqk_{l}"][:])
                qkT = [wp.tile([P, S], dt, name=f"qkT{mi}", tag=f"qkT{mi}") for mi in range(NHM)]
                for nj in range(NJ):
                    bs = 4 * (nj % 2)
                    psq = [pp.tile([P, 512], F32, name=f"psq{mi}", tag=f"b{bs + mi}")
                           for mi in range(NHM)]
                    for t, ki in enumerate(KORDER):
                        wk = wp.tile([P, 4 * DH], dt, name="wqk_t", tag="wqk_t", bufs=3)
                        nc.sync.dma_start(wk[:], ins[f"wqk_{l}"][ki * P:(ki + 1) * P, :])
                        for mi in range(NHM):
                            nc.tensor.matmul(psq[mi][:], wk[:, mi * P:(mi + 1) * P],
                                             yslice(ki, nj * 512, (nj + 1) * 512),
                                             start=(t == 0), stop=(t == KT - 1))
                    for mi in range(NHM):
                        nc.scalar.activation(qkT[mi][:, nj * 512:(nj + 1) * 512], psq[mi][:],
                                             AF.Identity, bias=bqk_pp[:, mi:mi + 1])
                # v token-major
                bv_row = wp.tile([1, 2 * DH], F32, name="bv_row", tag="bv_row")
                nc.sync.dma_start(bv_row[:], ins[f"bv_{l}"][:])
                bv_b = wp.tile([P, 2 * DH], F32, name="bv_b", tag="bv_b")
                pbv = pp.tile([P, 2 * DH], F32, name="pbv", tag="b4")
                nc.tensor.matmul(pbv[:], ones_rf[:], bv_row[:], start=True, stop=True)
                nc.scalar.copy(bv_b[:], pbv[:])
                v_sb = [wp.tile([P, 2 * DH], dt, name=f"v_sb{mi}", tag=f"v_sb{mi}") for mi in range(ST)]
                for vg in range(2):
                    bs = 4 * (vg % 2)
                    psv = [pp.tile([P, 2 * DH], F32, name=f"psv{mi}", tag=f"b{bs + mi}")
                           for mi in range(4)]
                    for t, ki in enumerate(KORDER):
                        wk = wp.tile([P, 2 * DH], dt, name="wv_t", tag="wv_t", bufs=3)
                        nc.sync.dma_start(wk[:], ins[f"wv_{l}"][ki * P:(ki + 1) * P, :])
                        for mi in range(4):
                            tb = vg * 4 + mi
                            nc.tensor.matmul(psv[mi][:], yslice(ki, tb * P, (tb + 1) * P), wk[:],
                                             start=(t == 0), stop=(t == KT - 1))
                    for mi in range(4):
                        nc.vector.tensor_add(v_sb[vg * 4 + mi][:], psv[mi][:], bv_b[:])
                for h in range(HPC):
                    qh, kh = qkT[h], qkT[HPC + h]
                    AT = [wp.tile([P, S], dt, name=f"AT{kb}", tag=f"AT{kb}") for kb in range(ST)]
                    for qb in range(ST):
                        sc = wp.tile([P, S], F32, name="sc", tag=f"sc{qb % 2}")
                        for nj in range(NJ):
                            pss = pp.tile([P, 512], F32, name="pss", tag=f"b{(qb % 2) * 2 + nj}")
                            nc.tensor.matmul(pss[:], qh[:, qb * P:(qb + 1) * P],
                                             kh[:, nj * 512:(nj + 1) * 512],
                                             start=True, stop=True)
                            nc.scalar.mul(sc[:, nj * 512:(nj + 1) * 512], pss[:],
                                          1.0 / float(np.sqrt(DH)))
                        nmax = wp.tile([P, 1], F32, name="nmax", tag="nmax", bufs=2)
                        nc.vector.reduce_max(nmax[:], sc[:], axis=AX, negate=True)
                        pr = wp.tile([P, S], dt, name="pr", tag=f"pr{qb % 2}")
                        nc.scalar.activation(pr[:], sc[:], AF.Exp, bias=nmax[:], scale=1.0)
                        for kb in range(ST):
                            tp = pp.tile([P, P], dt, name="tpa", tag=f"b{4 + kb % 2}")
                            nc.tensor.transpose(tp[:], pr[:, kb * P:(kb + 1) * P], ident(dt)[:])
                            if kb % 2 == 0:
                                nc.vector.tensor_copy(AT[kb][:, qb * P:(qb + 1) * P], tp[:])
                            else:
                                nc.scalar.copy(AT[kb][:, qb * P:(qb + 1) * P], tp[:])
                    ao = aoT[h]
                    for nj in range(NJ):
                        po = pp.tile([P, 512], F32, name="po", tag=f"b{6 + nj}")
                        for kb in range(ST):
                            nc.tensor.matmul(po[:], v_sb[kb][:, h * DH:(h + 1) * DH],
                                             AT[kb][:, nj * 512:(nj + 1) * 512],
                                             start=(kb == 0), stop=(kb == ST - 1))
                        nc.vector.tensor_copy(ao[:, nj * 512:(nj + 1) * 512], po[:])
                    # per-token softmax denominator: column-sums of A^T
                    srow = wp.tile([1, S], F32, name="srow_a", tag="srow_a")
                    for nj in range(NJ):
                        ps_s = pp.tile([1, 512], F32, name="ps_sr", tag="b4")
                        for kb in range(ST):
                            nc.tensor.matmul(ps_s[:], ones_c(dt)[:],
                                             AT[kb][:, nj * 512:(nj + 1) * 512],
                                             start=(kb == 0), stop=(kb == ST - 1))
                        nc.scalar.copy(srow[:, nj * 512:(nj + 1) * 512], ps_s[:])
                    rrow = wp.tile([1, S], F32, name="rrow", tag="rrow")
                    nc.vector.reciprocal(rrow[:], srow[:])
                    inv_b = wp.tile([P, S], F32, name="inv_b", tag="inv_b")
                    for nj in range(NJ):
                        pbc = pp.tile([P, 512], F32, name="pbc", tag="b5")
                        nc.tensor.matmul(pbc[:], ones_rf[:], rrow[:, nj * 512:(nj + 1) * 512],
                                         start=True, stop=True)
                        nc.scalar.copy(inv_b[:, nj * 512:(nj + 1) * 512], pbc[:])
                    nc.vector.tensor_mul(ao[:], ao[:], inv_b[:])

            # ---------- gate top-2 selection (gT computed in ln) ----------
            with tc.tile_pool(name=f"gate_{l}", bufs=1) as wp:
                sel = wp.tile([E, P], F32, name="sel", tag="sel")
                nc.sync.dma_start(sel[:], ins[f"sel_{l}"][:])
                ewT = wp.tile([E, S], F32, name="ewT", tag="ewT")
                for qb in range(ST):
                    tpg = pp.tile([P, E], F32, name="tpg", tag="b1")
                    nc.tensor.transpose(tpg[:], gT[:, qb * P:(qb + 1) * P], ident_f[:E, :E])
                    gtok = wp.tile([P, E], F32, name="gtok", tag="gtok", bufs=2)
                    nc.vector.tensor_copy(gtok[:], tpg[:])
                    nm1 = wp.tile([P, 1], F32, name="nm1", tag="nm1", bufs=2)
                    nc.vector.reduce_max(nm1[:], gtok[:], axis=AX, negate=True)
                    eg = wp.tile([P, E], F32, name="eg", tag="eg", bufs=2)
                    gs = wp.tile([P, 1], F32, name="gs", tag="gs", bufs=2)
                    nc.scalar.activation(eg[:], gtok[:], AF.Exp, bias=nm1[:], accum_out=gs[:])
                    rg_ = wp.tile([P, 1], F32, name="rg_", tag="rg_", bufs=2)
                    nc.vector.reciprocal(rg_[:], gs[:])
                    p_t = wp.tile([P, E], F32, name="p_t", tag="p_t", bufs=2)
                    nc.vector.tensor_scalar_mul(p_t[:], eg[:], rg_[:])
                    m1 = wp.tile([P, 1], F32, name="m1", tag="m1", bufs=2)
                    nc.vector.reduce_max(m1[:], p_t[:], axis=AX)
                    mask = wp.tile([P, E], F32, name="mask", tag="mask", bufs=2)
                    nc.vector.tensor_scalar(mask[:], p_t[:], m1[:], None, ALU.is_equal)
                    pmask = wp.tile([P, E], F32, name="pmask", tag="pmask", bufs=2)
                    nc.vector.scalar_tensor_tensor(pmask[:], mask[:], -1e30, p_t[:],
                                                   ALU.mult, ALU.add)
                    m2 = wp.tile([P, 1], F32, name="m2", tag="m2", bufs=2)
                    nc.vector.reduce_max(m2[:], pmask[:], axis=AX)
                    gemask = wp.tile([P, E], F32, name="gemask", tag="gemask", bufs=2)
                    nc.vector.tensor_scalar(gemask[:], p_t[:], m2[:], None, ALU.is_ge)
                    ew_t = wp.tile([P, E], F32, name="ew_t", tag="ew_t", bufs=2)
                    nc.vector.tensor_mul(ew_t[:], p_t[:], gemask[:])
                    tpe = pp.tile([E, P], F32, name="tpe", tag="b2")
                    nc.tensor.transpose(tpe[:], ew_t[:], ident_f[:])
                    nc.vector.tensor_copy(ewT[:, qb * P:(qb + 1) * P], tpe[:])
                for nj in range(NJ):
                    pe_ = pp.tile([P, 512], F32, name="pe_", tag="b3")
                    nc.tensor.matmul(pe_[:], sel[:], ewT[:, nj * 512:(nj + 1) * 512],
                                     start=True, stop=True)
                    nc.scalar.copy(ew_b[:, nj * 512:(nj + 1) * 512], pe_[:])

            if nc._dbg is not None and l == 1:
                nc.sync.dma_start(nc._dbg["ewb"][:, :], ew_b[:])
                nc.sync.dma_start(nc._dbg["ao0"][:, :], aoT[0][:])

            # ---------- MoE partials + RS + combine ----------
            with tc.tile_pool(name=f"moe_{l}", bufs=1) as wp:
                be_pp = wp.tile([P, KT], F32, name="be_pp", tag="be_pp")
                nc.sync.dma_start(be_pp[:], ins[f"be_{l}"][:])
                rs_ins = [dr.tile([D // 2, S], F32, name=f"rs1in{h}", tag=f"rs1in{h}_{l}")
                          for h in range(2)]
                rs_outs = [dr.tile([P, S], F32, name=f"rs1out{h}", tag=f"rs1out{h}_{l}")
                           for h in range(2)]
                we_v = ins[f"we_{l}"]
                G = 0
                for h in range(2):
                    for nj in range(NJ):
                        for g in range(2):
                            bs = 4 * (G % 2); G += 1
                            psz = [pp.tile([P, 512], F32, name=f"psz{mi}", tag=f"b{bs + mi}")
                                   for mi in range(4)]
                            for t, ki in enumerate(KORDER):
                                wk = wp.tile([P, 512], dt, name="we_t", tag="we_t", bufs=6)
                                nc.sync.dma_start(wk[:], we_v[ki * P:(ki + 1) * P,
                                                             h * 1024 + g * 512:
                                                             h * 1024 + (g + 1) * 512])
                                for mi in range(4):
                                    nc.tensor.matmul(psz[mi][:], wk[:, mi * P:(mi + 1) * P],
                                                     yslice(ki, nj * 512, (nj + 1) * 512),
                                                     start=(t == 0), stop=(t == KT - 1))
                            for mi in range(4):
                                gm = h * 8 + g * 4 + mi
                                z = wp.tile([P, 512], F32, name="z", tag="z", bufs=4)
                                nc.vector.scalar_tensor_tensor(z[:], psz[mi][:],
                                                               be_pp[:, gm:gm + 1],
                                                               ew_b[:, nj * 512:(nj + 1) * 512],
                                                               ALU.add, ALU.mult)
                                nc.sync.dma_start(rs_ins[h][(g * 4 + mi) * P:(g * 4 + mi + 1) * P,
                                                            nj * 512:(nj + 1) * 512], z[:])
                    nc.gpsimd.collective_compute("ReduceScatter", ALU.add, replica_groups=RG,
                                                 ins=[rs_ins[h].opt()], outs=[rs_outs[h].opt()])
                for fi in range(FT):
                    r = wp.tile([P, S], F32, name=f"r1_{fi}", tag=f"r1_{fi}")
                    nc.sync.dma_start(r[:], rs_outs[fi][:])
                    nc.vector.tensor_add(r[:], r[:], aoT[fi][:])
                    nc.vector.tensor_add(x2_sl[fi][:], r[:], y_sl[fi][:])

            if nc._dbg is not None and l == 1:
                nc.sync.dma_start(nc._dbg["x20"][:, :], x2_sl[0][:])

            # ---------- LN2 (writes tT into yT tiles) ----------
            ln(x2_sl, None, None, yT, None, f"ln2_{l}", l)

            if nc._dbg is not None and l == 1:
                nc.sync.dma_start(nc._dbg["tT0"][:, :], yT[0][:].bitcast(F32))

            # ---------- FFN + RS ----------
            with tc.tile_pool(name=f"ffn_{l}", bufs=1) as wp:
                b1_pp = wp.tile([P, UT], F32, name="b1_pp", tag="b1_pp")
                nc.sync.dma_start(b1_pp[:], ins[f"b1_{l}"][:])
                b2_pp = wp.tile([P, FT], F32, name="b2_pp", tag="b2_pp")
                nc.sync.dma_start(b2_pp[:], ins[f"b2_{l}"][:])
                rs2_ins = [dr.tile([D // 2, S], F32, name=f"rs2in{h}", tag=f"rs2in{h}_{l}")
                           for h in range(2)]
                rs2_outs = [dr.tile([P, S], F32, name=f"rs2out{h}", tag=f"rs2out{h}_{l}")
                            for h in range(2)]
                # ffn1 -> u (full-S tiles)
                u = [wp.tile([P, S], dt, name=f"u{mi}", tag=f"u{mi}") for mi in range(UT)]
                G = 0
                for nj in range(NJ):
                    for g in range(2):
                        bs = 4 * (G % 2); G += 1
                        psu = [pp.tile([P, 512], F32, name=f"psu{mi}", tag=f"b{bs + mi}")
                               for mi in range(4)]
                        for t, ki in enumerate(KORDER):
                            wk = wp.tile([P, 512], dt, name="w1_t", tag="w1_t", bufs=4)
                            nc.sync.dma_start(wk[:], ins[f"w1_{l}"][ki * P:(ki + 1) * P,
                                                                    g * 512:(g + 1) * 512])
                            for mi in range(4):
                                nc.tensor.matmul(psu[mi][:], wk[:, mi * P:(mi + 1) * P],
                                                 yslice(ki, nj * 512, (nj + 1) * 512),
                                                 start=(t == 0), stop=(t == KT - 1))
                        for mi in range(4):
                            um = g * 4 + mi
                            nc.scalar.activation(u[um][:, nj * 512:(nj + 1) * 512], psu[mi][:],
                                                 AF.Gelu_apprx_tanh, bias=b1_pp[:, um:um + 1])
                # ffn2: halves over output features, split RS
                w2_v = ins[f"w2_{l}"]
                for h in range(2):
                    for nj in range(NJ):
                        for g in range(2):
                            bs = 4 * (G % 2); G += 1
                            psf = [pp.tile([P, 512], F32, name=f"psf{mi}", tag=f"b{bs + mi}")
                                   for mi in range(4)]
                            for ki in range(UT):
                                wk2 = wp.tile([P, 512], dt, name="w2_t", tag="w2_t", bufs=6)
                                nc.sync.dma_start(wk2[:], w2_v[ki * P:(ki + 1) * P,
                                                               h * 1024 + g * 512:
                                                               h * 1024 + (g + 1) * 512])
                                for mi in range(4):
                                    nc.tensor.matmul(psf[mi][:], wk2[:, mi * P:(mi + 1) * P],
                                                     u[ki][:, nj * 512:(nj + 1) * 512],
                                                     start=(ki == 0), stop=(ki == UT - 1))
                            for mi in range(4):
                                zf = wp.tile([P, 512], F32, name="zf", tag="zf", bufs=4)
                                nc.scalar.copy(zf[:], psf[mi][:])
                                nc.sync.dma_start(rs2_ins[h][(g * 4 + mi) * P:(g * 4 + mi + 1) * P,
                                                             nj * 512:(nj + 1) * 512], zf[:])
                    nc.gpsimd.collective_compute("ReduceScatter", ALU.add, replica_groups=RG,
                                                 ins=[rs2_ins[h].opt()], outs=[rs2_outs[h].opt()])
                for fi in range(FT):
                    r2 = wp.tile([P, S], F32, name=f"r2_{fi}", tag=f"r2_{fi}")
                    nc.sync.dma_start(r2[:], rs2_outs[fi][:])
                    nc.vector.tensor_scalar_add(x_sl[fi][:], r2[:], b2_pp[:, fi:fi + 1])
                if nc._dbg is not None and l == 1:
                    nc.sync.dma_start(nc._dbg["xsl0"][:, :], x_sl[1][:])

        # =============== final projection (vocab-split, fp32r) ===============
        with tc.tile_pool(name="finp", bufs=1) as wp:
            ag_ins = [dr.tile([P, S], F32, name=f"agf_in{h}", tag=f"agf_in{h}")
                      for h in range(FT)]
            ag_outs = [dr.tile([D // FT, S], F32, name=f"agf_out{h}", tag=f"agf_out{h}",
                               addr_space="Shared") for h in range(FT)]
            for fi in range(FT):
                nc.sync.dma_start(ag_ins[fi][:], x_sl[fi][:])
                nc.gpsimd.collective_compute("AllGather", ALU.bypass, replica_groups=RG,
                                             ins=[ag_ins[fi].opt()], outs=[ag_outs[fi].opt()])
            xfT = [act.tile([P, S], F32R, name=f"xfT{ki}", tag=f"yT{ki}") for ki in range(KT)]
            for ki in KORDER:
                h, r = ki // ST, ki % ST
                nc.gpsimd.dma_start(xfT[ki][:], ag_outs[h][r * P:(r + 1) * P, :])
            if nc._dbg is not None:
                nc.sync.dma_start(nc._dbg["yT0"][:, :], xfT[0][:].bitcast(F32))
                nc.sync.dma_start(nc._dbg["yT8"][:, :], xfT[8][:].bitcast(F32))
                nc.sync.dma_start(nc._dbg["u0"][:, :], ag_outs[1][0:P, :])
            bo_row = wp.tile([1, VS], F32, name="bo_row", tag="bo_row")
            nc.sync.dma_start(bo_row[:], bo[:])
            ob_b = wp.tile([P, VS], F32, name="ob_b", tag="ob_b")
            for vb in range(8):
                pb = pp.tile([P, 500], F32, name="pb", tag=f"b{vb}")
                nc.tensor.matmul(pb[:], ones_rf[:], bo_row[:, vb * 500:(vb + 1) * 500],
                                 start=True, stop=True)
                nc.scalar.copy(ob_b[:, vb * 500:(vb + 1) * 500], pb[:])
            G = 0
            for nj in range(8):
                wks = []
                for ki in range(KT):
                    wk = wp.tile([P, 500], F32R, name="wo_k", tag="wo_k", bufs=KT + 2)
                    nc.sync.dma_start(wk[:], wo[ki * P:(ki + 1) * P, nj * 500:(nj + 1) * 500])
                    wks.append(wk)
                for g in range(2):
                    bs = 4 * (G % 2); G += 1
                    psums = [pp.tile([P, 500], F32, name=f"po{mi}", tag=f"b{bs + mi}")
                             for mi in range(4)]
                    for t, ki in enumerate(KORDER):
                        for mi in range(4):
                            tb = g * 4 + mi
                            nc.tensor.matmul(psums[mi][:], xfT[ki][:, tb * P:(tb + 1) * P],
                                             wks[ki][:], start=(t == 0), stop=(t == KT - 1))
                    for mi in range(4):
                        tb = g * 4 + mi
                        lo = wp.tile([P, 500], F32, name="lo", tag="lo", bufs=4)
                        nc.vector.tensor_add(lo[:], psums[mi][:], ob_b[:, nj * 500:(nj + 1) * 500])
                        nc.sync.dma_start(out[tb * P:(tb + 1) * P, nj * 500:(nj + 1) * 500], lo[:])


def kernel(**inputs):
    if "nc" not in _CACHE:
        _CACHE["nc"] = build_nc()
    nc = _CACHE["nc"]
    in_maps = prepare_in_maps(inputs)
    r = run_bass_kernel_spmd(nc, in_maps, core_ids=list(range(NC)), trace=False)
    logits = np.concatenate([r.results[c]["out"] for c in range(NC)], axis=1)
    return logits.reshape(B, S, V).astype(np.float32)

